# revision 1
# baseline (speedup 1.0000x reference)
"""Trainium2 Bass kernel for nn_ContrastiveLoss (8-core SPMD).

Strategy (memory-bound gather problem):
  - Shard the 262144 pos + 262144 neg pairs across 8 NeuronCores
    (32768 + 32768 pairs per core); replicate Xemb (4MB, stays in HBM).
  - Per core, per batch of 2048 pairs: two SWDGE dma_gather calls pull the
    512B embedding rows for i and j into SBUF [128, 16, 128] tiles (row t of
    the batch lands on partition t%128, chunk t//128).
  - DVE: diff = A - B.  ACT: square (for pos batches with fused per-partition
    accumulation -> one f32 per batch per partition).  DVE: per-pair reduce
    of the squared diff over D=128 for neg batches.
  - Device output per core: [128, 272] f32 = 16 pos batch-accum columns +
    256 per-pair d^2 columns for the 32768 neg pairs.
  - Host: f64 final math - pos mean, and relu(softplus(h_bias) - sqrt(d^2))^2
    mean for neg (elementwise, order-independent, so layout doesn't matter).
"""

import sys

if "/opt/trn_rl_repo" not in sys.path:
    sys.path.insert(0, "/opt/trn_rl_repo")

import numpy as np

import concourse.bass as bass
import concourse.mybir as mybir
from concourse.library_config import mlp
from concourse.library_overlay import lower_extended_insts
from concourse.bass_utils import run_bass_kernel_spmd

# Problem shapes (hardcoded per the harness contract).
N, D = 8192, 128
NUM_PAIRS = 262144
NCORES = 8
PAIRS = NUM_PAIRS // NCORES      # 32768 pairs per core per loss type
B = 1024                         # pairs per gather batch (1024 = single-packet max)
CH = B // 128                    # 16 chunks per batch
NB = PAIRS // B                  # 16 batches per loss type
NBT = 2 * NB                     # 32 batches total (pos then neg)
SLOTS_G = 4                      # gather tile slots (A/B pools)
SLOTS_C = 3                      # diff/square tile slots
R_COLS = NB + NB * CH            # 16 pos accum cols + 256 neg per-pair cols

_nc_cache = None
_last_results = None             # stashed BassKernelResults (for test harness)


def _build_nc(reps=1):
    nc = bass.Bass()
    xemb = nc.dram_tensor("xemb", [N, D], mybir.dt.float32, kind="ExternalInput")
    idx = nc.dram_tensor(
        "idx", [4, 128, PAIRS // 16], mybir.dt.int16, kind="ExternalInput"
    )
    out = nc.dram_tensor("out", [128, R_COLS], mybir.dt.float32, kind="ExternalOutput")

    with (
        nc.sbuf_tensor("idx_sb", [128, 4, PAIRS // 16], mybir.dt.int16) as idx_sb,
        nc.sbuf_tensor("ga", [128, SLOTS_G, CH, D], mybir.dt.float32) as ga,
        nc.sbuf_tensor("gb", [128, SLOTS_G, CH, D], mybir.dt.float32) as gb,
        nc.sbuf_tensor("df", [128, SLOTS_C, CH, D], mybir.dt.float32) as df,
        nc.sbuf_tensor("sq", [128, SLOTS_C, CH, D], mybir.dt.float32) as sq,
        nc.sbuf_tensor("acc", [128, R_COLS], mybir.dt.float32) as acc,
        nc.semaphore("s_idx") as s_idx,
        nc.semaphore("s_ga0") as s_ga0,
        nc.semaphore("s_ga1") as s_ga1,
        nc.semaphore("s_ga2") as s_ga2,
        nc.semaphore("s_ga3") as s_ga3,
        nc.semaphore("s_gb0") as s_gb0,
        nc.semaphore("s_gb1") as s_gb1,
        nc.semaphore("s_gb2") as s_gb2,
        nc.semaphore("s_gb3") as s_gb3,
        nc.semaphore("s_sub") as s_sub,
        nc.semaphore("s_act") as s_act,
        nc.semaphore("s_red") as s_red,
        nc.semaphore("s_out") as s_out,
        nc.Block() as block,
    ):
        s_ga = [s_ga0, s_ga1, s_ga2, s_ga3]
        s_gb = [s_gb0, s_gb1, s_gb2, s_gb3]

        @block.sync
        def _(sync):
            for plane in range(4):
                sync.dma_start(idx_sb[:, plane, :], idx[plane]).then_inc(s_idx, 16)
            sync.wait_ge(s_act, reps * NBT)
            sync.wait_ge(s_red, reps * NB)
            sync.dma_start(out[:], acc[:]).then_inc(s_out, 16)
            sync.wait_ge(s_out, 16)

        @block.gpsimd
        def _(gpsimd):
            gpsimd.load_library(mlp)
            nreg = gpsimd.to_reg(B)
            gpsimd.wait_ge(s_idx, 64)
            for kg in range(reps * NBT):
                k = kg % NBT
                s = kg % SLOTS_G
                loss, b = divmod(k, NB)
                ip, jp = 2 * loss, 2 * loss + 1
                cols = slice(b * (B // 16), (b + 1) * (B // 16))
                if kg >= SLOTS_G:
                    gpsimd.wait_ge(s_sub, kg - SLOTS_G + 1)
                gpsimd.dma_gather(
                    ga[:, s], xemb[:], idx_sb[:, ip, cols], B, nreg, D,
                ).then_inc(s_ga[s], 16)
                gpsimd.dma_gather(
                    gb[:, s], xemb[:], idx_sb[:, jp, cols], B, nreg, D,
                ).then_inc(s_gb[s], 16)
            del k, s, loss, b, ip, jp, cols

        @block.vector
        def _(vector):
            def emit_reduce(kgp):
                # per-pair reduce for neg batch kgp: [128, CH, D] -> [128, CH]
                sc = kgp % SLOTS_C
                bn = (kgp % NBT) - NB
                vector.wait_ge(s_act, kgp + 1)
                vector.tensor_reduce(
                    acc[:, NB + bn * CH : NB + (bn + 1) * CH],
                    sq[:, sc],
                    axis=mybir.AxisListType.X,
                    op=mybir.AluOpType.add,
                ).then_inc(s_red, 1)

            for kg in range(reps * NBT):
                k = kg % NBT
                s = kg % SLOTS_G
                sc = kg % SLOTS_C
                if kg >= SLOTS_C:
                    vector.wait_ge(s_act, kg - SLOTS_C + 1)
                vector.wait_ge(s_ga[s], 16 * (kg // SLOTS_G + 1))
                vector.wait_ge(s_gb[s], 16 * (kg // SLOTS_G + 1))
                vector.tensor_sub(df[:, sc], ga[:, s], gb[:, s]).then_inc(s_sub, 1)
                if (kg - 1) % NBT >= NB and kg >= 1:
                    emit_reduce(kg - 1)
            emit_reduce(reps * NBT - 1)

        @block.scalar
        def _(scalar):
            nred = 0
            for kg in range(reps * NBT):
                k = kg % NBT
                sc = kg % SLOTS_C
                scalar.wait_ge(s_sub, kg + 1)
                if kg >= SLOTS_C and (kg - SLOTS_C) % NBT >= NB:
                    nred += 1
                    scalar.wait_ge(s_red, nred)
                if k < NB:
                    scalar.activation(
                        sq[:, sc],
                        df[:, sc],
                        mybir.ActivationFunctionType.Square,
                        accum_out=acc[:, k : k + 1],
                    ).then_inc(s_act, 1)
                else:
                    scalar.activation(
                        sq[:, sc],
                        df[:, sc],
                        mybir.ActivationFunctionType.Square,
                    ).then_inc(s_act, 1)

    lower_extended_insts(nc)
    return nc


def _get_nc():
    global _nc_cache
    if _nc_cache is None:
        _nc_cache = _build_nc()
    return _nc_cache


def _wrap_idx(arr):
    """int32 [PAIRS] -> wrapped int16 [128, PAIRS//16] for dma_gather."""
    wrapped = arr.astype(np.int16).reshape(PAIRS // 16, 16).T  # [16, PAIRS//16]
    return np.tile(wrapped, (8, 1))


def kernel(**inputs):
    global _last_results
    Xemb = np.ascontiguousarray(np.asarray(inputs["Xemb"], dtype=np.float32))
    h_bias = float(np.asarray(inputs["h_bias"]))
    pos_idx = np.asarray(inputs["pos_idx"], dtype=np.int32)
    neg_idx = np.asarray(inputs["neg_idx"], dtype=np.int32)

    in_maps = []
    for c in range(NCORES):
        sl = slice(c * PAIRS, (c + 1) * PAIRS)
        planes = np.stack(
            [
                _wrap_idx(pos_idx[sl, 0]),
                _wrap_idx(pos_idx[sl, 1]),
                _wrap_idx(neg_idx[sl, 0]),
                _wrap_idx(neg_idx[sl, 1]),
            ]
        )
        in_maps.append({"xemb": Xemb, "idx": planes})

    res = run_bass_kernel_spmd(_get_nc(), in_maps, core_ids=list(range(NCORES)))
    _last_results = res

    pos_sum = 0.0
    neg_parts = []
    for c in range(NCORES):
        o = np.asarray(res.results[c]["out"], dtype=np.float64)
        pos_sum += o[:, :NB].sum()
        neg_parts.append(o[:, NB:])
    neg_sq = np.concatenate(neg_parts, axis=1).ravel()  # all 262144 neg d^2

    bias = np.logaddexp(0.0, h_bias)  # softplus, f64
    pos_loss = 0.5 * pos_sum / NUM_PAIRS
    d = np.sqrt(np.maximum(neg_sq, 0.0))
    m = np.maximum(bias - d, 0.0)
    neg_loss = 0.5 * np.mean(m * m)
    return np.array([pos_loss, neg_loss], dtype=np.float32)



# revision 10
# speedup vs baseline: 3.8572x; 3.8572x over previous
"""Trainium2 Bass kernel for nn_ContrastiveLoss (8-core SPMD).

Gather-free formulation. The baseline gathered two 512B embedding rows per
pair via SWDGE dma_gather; descriptor generation on GPSIMD (~8ns/descriptor
x 131072 descriptors/core) made it ~1.1ms. Instead, compute the FULL
8192x8192 pairwise distance matrix blockwise on the PE array and contract
it against dense pair-count matrices built host-side from the indices:

  pos_loss_sum = sum_{n,m} Cpos[n,m] * d2[n,m]
  neg_loss_sum = sum_{n,m} Cneg[n,m] * relu(bias - sqrt(d2[n,m]))^2

Each core owns a 1024-row block of d2 (8 partition-tiles of 128 rows):
  - PE: psum = Xblk^T X (K=128, bf16) accumulated with a K=1 matmul adding
    -0.5*nrm_n, so  -2*psum = -2G + nrm_n.
  - ACT: d2c = Relu(-2*psum + nrm_m)  (bias = per-partition nrm, fused clamp
    that guards sqrt against bf16-rounded negative diagonal cells)
  - ACT: d = Sqrt(d2c); m = Relu(-d + softplus_bias); m2 = Square(m)
  - DVE: tensor_tensor_reduce  acc_pos += sum(d2c * Cpos_tile),
                               acc_neg += sum(m2 * Cneg_tile)
  - Cpos/Cneg tiles stream from HBM (bf16, 32MB/core) double-buffered.
Host: build Cpos/Cneg with np.add.at (index-only preprocessing), final
f64 mean + 0.5 factors. Counts <= ~3 are exact in bf16; C_ii = 0 for both
pair types, so clamped diagonal cells never contribute.
"""

import sys

if "/opt/trn_rl_repo" not in sys.path:
    sys.path.insert(0, "/opt/trn_rl_repo")

import numpy as np
import ml_dtypes

import concourse.bass as bass
import concourse.mybir as mybir
from concourse.library_overlay import lower_extended_insts
from concourse.bass_utils import run_bass_kernel_spmd

N, D = 8192, 128
NUM_PAIRS = 262144
NCORES = 8
ROWS = N // NCORES            # 1024 rows of d2 per core
NPT = ROWS // 128             # 8 partition-tiles per core
NCH = 2048                    # columns per chunk
NCHUNK = N // NCH             # 4 chunks per partition-tile
NIT = NPT * NCHUNK            # 32 iterations per core
CSLOT = 3                     # C-tile stream slots

BF16 = ml_dtypes.bfloat16

_nc_cache = None
_last_results = None


def _build_nc():
    nc = bass.Bass()
    f32 = mybir.dt.float32
    bf = mybir.dt.bfloat16
    xt = nc.dram_tensor("xt", [128, N], bf, kind="ExternalInput")
    nrow = nc.dram_tensor("nrow", [1, N], bf, kind="ExternalInput")  # -0.5*nrm
    nrm8 = nc.dram_tensor("nrm8", [128, NPT], f32, kind="ExternalInput")
    ones1 = nc.dram_tensor("ones1", [1, 128], bf, kind="ExternalInput")
    xl = nc.dram_tensor("xl", [128, ROWS], bf, kind="ExternalInput")
    biasv = nc.dram_tensor("biasv", [128, 1], f32, kind="ExternalInput")
    cpos = nc.dram_tensor("cpos", [ROWS, N], bf, kind="ExternalInput")
    cneg = nc.dram_tensor("cneg", [ROWS, N], bf, kind="ExternalInput")
    out = nc.dram_tensor("out", [128, 2 * NIT], f32, kind="ExternalOutput")

    from contextlib import ExitStack

    with ExitStack() as ctx:
        xt_sb = ctx.enter_context(nc.sbuf_tensor("xt_sb", [128, N], bf))
        nrow_sb = ctx.enter_context(nc.sbuf_tensor("nrow_sb", [1, N], bf))
        nrm8_sb = ctx.enter_context(nc.sbuf_tensor("nrm8_sb", [128, NPT], f32))
        ones_sb = ctx.enter_context(nc.sbuf_tensor("ones_sb", [1, 128], bf))
        xl_sb = ctx.enter_context(nc.sbuf_tensor("xl_sb", [128, ROWS], bf))
        bias_sb = ctx.enter_context(nc.sbuf_tensor("bias_sb", [128, 1], f32))
        cp_sb = ctx.enter_context(nc.sbuf_tensor("cp_sb", [128, CSLOT, NCH], bf))
        cn_sb = ctx.enter_context(nc.sbuf_tensor("cn_sb", [128, CSLOT, NCH], bf))
        d2c = ctx.enter_context(nc.sbuf_tensor("d2c", [128, 2, NCH], bf))
        dd = ctx.enter_context(nc.sbuf_tensor("dd", [128, 2, NCH], bf))
        mm = ctx.enter_context(nc.sbuf_tensor("mm", [128, 2, NCH], bf))
        m2 = ctx.enter_context(nc.sbuf_tensor("m2", [128, 2, NCH], bf))
        junk = ctx.enter_context(nc.sbuf_tensor("junk", [128, NCH], bf))
        acc = ctx.enter_context(nc.sbuf_tensor("acc", [128, 2 * NIT], f32))
        ps = ctx.enter_context(nc.psum_tensor("ps", [128, 2, NCH], f32))
        s_in = ctx.enter_context(nc.semaphore("s_in"))
        s_cs = [ctx.enter_context(nc.semaphore(f"s_c{i}")) for i in range(CSLOT)]
        s_mm = ctx.enter_context(nc.semaphore("s_mm"))
        s_t = ctx.enter_context(nc.semaphore("s_t"))
        s_sq = ctx.enter_context(nc.semaphore("s_sq"))
        s_ttr = ctx.enter_context(nc.semaphore("s_ttr"))
        s_out = ctx.enter_context(nc.semaphore("s_out"))
        s_sa = ctx.enter_context(nc.semaphore("s_sa"))
        s_sv = ctx.enter_context(nc.semaphore("s_sv"))
        block = ctx.enter_context(nc.Block())

        @block.sync
        def _(sync):
            sync.dma_start(xt_sb[:], xt[:]).then_inc(s_in, 16)
            sync.dma_start(nrow_sb[:], nrow[:]).then_inc(s_in, 16)
            sync.dma_start(nrm8_sb[:], nrm8[:]).then_inc(s_in, 16)
            sync.dma_start(ones_sb[:], ones1[:]).then_inc(s_in, 16)
            sync.dma_start(bias_sb[:], biasv[:]).then_inc(s_in, 16)
            sync.dma_start(xl_sb[:], xl[:]).then_inc(s_in, 16)
            for k in range(NIT):
                pt, ch = divmod(k, NCHUNK)
                s = k % CSLOT
                rs = slice(pt * 128, (pt + 1) * 128)
                cs = slice(ch * NCH, (ch + 1) * NCH)
                if k >= CSLOT:
                    sync.wait_ge(s_ttr, k - CSLOT + 1)
                sync.dma_start(cp_sb[:, s], cpos[rs, cs]).then_inc(s_cs[s], 16)
                sync.dma_start(cn_sb[:, s], cneg[rs, cs]).then_inc(s_cs[s], 16)
            sync.wait_ge(s_ttr, NIT)
            sync.dma_start(out[:], acc[:]).then_inc(s_out, 16)
            sync.wait_ge(s_out, 16)

        @block.tensor
        def _(tensor):
            tensor.wait_ge(s_in, 16 * 6)
            for k in range(NIT):
                pt, ch = divmod(k, NCHUNK)
                r = k % 2
                if k >= 2:
                    tensor.wait_ge(s_t, k - 1)
                for c in range(NCH // 512):
                    c0 = ch * NCH + c * 512
                    tensor.matmul(
                        ps[:, r, c * 512 : (c + 1) * 512],
                        xl_sb[:, pt * 128 : (pt + 1) * 128],
                        xt_sb[:, c0 : c0 + 512],
                        start=True,
                        stop=False,
                    )
                    tensor.matmul(
                        ps[:, r, c * 512 : (c + 1) * 512],
                        ones_sb[:, :],
                        nrow_sb[:, c0 : c0 + 512],
                        start=False,
                        stop=True,
                    ).then_inc(s_mm, 1)

        @block.scalar
        def _(scalar):
            nmm = NCH // 512
            for k in range(NIT):
                pt, ch = divmod(k, NCHUNK)
                r = k % 2
                if k >= 2:
                    scalar.wait_ge(s_ttr, k - 1)
                scalar.wait_ge(s_mm, nmm * min(NIT, k + 2))
                scalar.activation(
                    d2c[:, r],
                    ps[:, r],
                    mybir.ActivationFunctionType.Relu,
                    bias=nrm8_sb[:, pt : pt + 1],
                    scale=-2.0,
                ).then_inc(s_t, 1)
                scalar.wait_ge(s_t, k + 1)
                scalar.activation(
                    dd[:, r], d2c[:, r], mybir.ActivationFunctionType.Sqrt
                ).then_inc(s_sa, 1)
                scalar.wait_ge(s_sa, 2 * k + 1)
                scalar.activation(
                    mm[:, r],
                    dd[:, r],
                    mybir.ActivationFunctionType.Relu,
                    bias=bias_sb[:, 0:1],
                    scale=-1.0,
                ).then_inc(s_sa, 1)
                scalar.wait_ge(s_sa, 2 * k + 2)
                scalar.activation(
                    m2[:, r], mm[:, r], mybir.ActivationFunctionType.Square
                ).then_inc(s_sq, 1)

        @block.vector
        def _(vector):
            for k in range(NIT):
                r = k % 2
                s = k % CSLOT
                if k >= 1:
                    vector.wait_ge(s_ttr, k)
                vector.wait_ge(s_cs[k % CSLOT], 32 * (k // CSLOT + 1))
                vector.wait_ge(s_t, k + 1)
                vector.tensor_tensor(
                    junk[:], d2c[:, r], cp_sb[:, s], op=mybir.AluOpType.mult
                ).then_inc(s_sv, 1)
                vector.wait_ge(s_sv, 3 * k + 1)
                vector.tensor_reduce(
                    acc[:, k : k + 1],
                    junk[:],
                    axis=mybir.AxisListType.X,
                    op=mybir.AluOpType.add,
                ).then_inc(s_sv, 1)
                vector.wait_ge(s_sv, 3 * k + 2)
                vector.wait_ge(s_sq, k + 1)
                vector.tensor_tensor(
                    junk[:], m2[:, r], cn_sb[:, s], op=mybir.AluOpType.mult
                ).then_inc(s_sv, 1)
                vector.wait_ge(s_sv, 3 * k + 3)
                vector.tensor_reduce(
                    acc[:, NIT + k : NIT + k + 1],
                    junk[:],
                    axis=mybir.AxisListType.X,
                    op=mybir.AluOpType.add,
                ).then_inc(s_ttr, 1)

    lower_extended_insts(nc)
    return nc


def _get_nc():
    global _nc_cache
    if _nc_cache is None:
        _nc_cache = _build_nc()
    return _nc_cache


def kernel(**inputs):
    global _last_results
    X = np.ascontiguousarray(np.asarray(inputs["Xemb"], dtype=np.float32))
    h_bias = float(np.asarray(inputs["h_bias"]))
    pos_idx = np.asarray(inputs["pos_idx"], dtype=np.int64)
    neg_idx = np.asarray(inputs["neg_idx"], dtype=np.int64)

    nrm = (X.astype(np.float64) ** 2).sum(axis=1)  # [N]
    xt_bf = np.ascontiguousarray(X.T).astype(BF16)  # [128, N]
    nrow_bf = (-0.5 * nrm[None, :]).astype(BF16)  # [1, N]
    ones_bf = np.ones((1, 128), dtype=BF16)
    softplus = float(np.logaddexp(0.0, h_bias))
    bias_col = np.full((128, 1), softplus, dtype=np.float32)

    cpos = np.zeros((N, N), dtype=np.float32)
    np.add.at(cpos, (pos_idx[:, 0], pos_idx[:, 1]), 1.0)
    cpos = cpos.astype(BF16)
    cneg = np.zeros((N, N), dtype=np.float32)
    np.add.at(cneg, (neg_idx[:, 0], neg_idx[:, 1]), 1.0)
    cneg = cneg.astype(BF16)

    in_maps = []
    for c in range(NCORES):
        rs = slice(c * ROWS, (c + 1) * ROWS)
        nrm8 = np.ascontiguousarray(
            nrm[rs].astype(np.float32).reshape(NPT, 128).T
        )  # [128, NPT]
        in_maps.append(
            {
                "xt": xt_bf,
                "xl": np.ascontiguousarray(xt_bf[:, rs]),
                "nrow": nrow_bf,
                "nrm8": nrm8,
                "ones1": ones_bf,
                "biasv": bias_col,
                "cpos": np.ascontiguousarray(cpos[rs]),
                "cneg": np.ascontiguousarray(cneg[rs]),
            }
        )

    res = run_bass_kernel_spmd(_get_nc(), in_maps, core_ids=list(range(NCORES)))
    _last_results = res

    pos_sum = 0.0
    neg_sum = 0.0
    for c in range(NCORES):
        o = np.asarray(res.results[c]["out"], dtype=np.float64)
        pos_sum += o[:, :NIT].sum()
        neg_sum += o[:, NIT:].sum()

    pos_loss = 0.5 * pos_sum / NUM_PAIRS
    neg_loss = 0.5 * neg_sum / NUM_PAIRS
    return np.array([pos_loss, neg_loss], dtype=np.float32)
